# revision 4
# baseline (speedup 1.0000x reference)
"""Trainium2 Bass kernel for 2-level 3D Haar DWT + LIF spiking (nn_DWT_3D).

Input:  x (2, 32, 64, 128, 128) fp32  =  (N, C, D, H, W)
Output: (sub2_0, high1, high2)
  sub2_0: (2, 32, 16, 32, 32)
  high1:  (2, 7, 32, 32, 64, 64)
  high2:  (2, 7, 32, 16, 32, 32)

Strategy: data-parallel over the N*C = 64 (n, c) slices, 8 per NeuronCore.
Per core, a depth-major stream: PE does the Haar H/W transforms as matmuls
(stage-1 H-matrix carries the 1/(sqrt(2)*tau) constant; the spike-domain
matmuls use exact +-1 bf16 matrices with the scale folded into the LIF
thresholds), DVE runs the sequential LIF recurrence, GPSIMD computes the
spike comparisons, ACT applies the output scaling.
"""
import numpy as np

N, C, D, H, W = 2, 32, 64, 128, 128
NCORES = 8
B = (N * C) // NCORES          # 8 (n, c) slices per core
E1 = D // 2                    # 32
E2 = D // 4                    # 16

TAU = 1.75
VTH = 0.35
A = float(np.float32(1.0 - 1.0 / TAU))
C1 = float(np.float32(0.7071067811865476 / TAU))
C3 = float(np.float32(0.7071067811865476))
TH2 = float(np.float32(VTH * TAU * np.sqrt(2.0)))

_CACHE = {}


def _constants():
    import ml_dtypes
    bf16 = ml_dtypes.bfloat16
    M0T = np.zeros((128, 128), np.float32)   # [h, p]
    for p in range(64):
        M0T[2 * p, p] = C1
        M0T[2 * p + 1, p] = C1
        M0T[2 * p, 64 + p] = C1
        M0T[2 * p + 1, 64 + p] = -C1
    Mw = np.zeros((128, 128), bf16)          # [w, q]
    for q in range(64):
        Mw[2 * q, q] = 1.0
        Mw[2 * q + 1, q] = 1.0
        Mw[2 * q, 64 + q] = 1.0
        Mw[2 * q + 1, 64 + q] = -1.0
    M02 = np.zeros((64, 64), bf16)           # [p, p2]
    for p2 in range(32):
        M02[2 * p2, p2] = 1.0
        M02[2 * p2 + 1, p2] = 1.0
        M02[2 * p2, 32 + p2] = 1.0
        M02[2 * p2 + 1, 32 + p2] = -1.0
    Mw2 = np.zeros((64, 64), bf16)           # [q, q2]
    for q2 in range(32):
        Mw2[2 * q2, q2] = 1.0
        Mw2[2 * q2 + 1, q2] = 1.0
        Mw2[2 * q2, 32 + q2] = 1.0
        Mw2[2 * q2 + 1, 32 + q2] = -1.0
    ident = np.eye(64, dtype=bf16)
    return {"m0t": M0T, "mw": Mw, "m02": M02, "mw2": Mw2, "ident": ident}


def _build():
    import concourse.bacc as bacc
    import concourse.bass as bass
    import concourse.tile as tile
    import concourse.mybir as mybir

    dt = mybir.dt
    Alu = mybir.AluOpType
    Act = mybir.ActivationFunctionType
    ts = bass.ts

    nc = bacc.Bacc("TRN2", target_bir_lowering=False, debug=False,
                   num_devices=NCORES)

    x_d = nc.dram_tensor("x", [B, D, H, W], dt.float32, kind="ExternalInput")
    m0t_d = nc.dram_tensor("m0t", [128, 128], dt.float32, kind="ExternalInput")
    mw_d = nc.dram_tensor("mw", [128, 128], dt.bfloat16, kind="ExternalInput")
    m02_d = nc.dram_tensor("m02", [64, 64], dt.bfloat16, kind="ExternalInput")
    mw2_d = nc.dram_tensor("mw2", [64, 64], dt.bfloat16, kind="ExternalInput")
    id_d = nc.dram_tensor("ident", [64, 64], dt.bfloat16, kind="ExternalInput")
    out1_d = nc.dram_tensor("out1", [B, E1, 2, 128, 128], dt.float32,
                            kind="ExternalOutput")
    out2_d = nc.dram_tensor("out2", [B, E2, 2, 64, 64], dt.float32,
                            kind="ExternalOutput")

    FB = B * 128    # 1024: stage-1 free width (all 8 slices batched)
    FS = B * 64     # 512:  stage-2 free width

    from contextlib import ExitStack
    with tile.TileContext(nc) as tc, ExitStack() as ctx:
        consts = ctx.enter_context(tc.tile_pool(name="consts", bufs=1))
        state = ctx.enter_context(tc.tile_pool(name="state", bufs=1))
        xin = ctx.enter_context(tc.tile_pool(name="xin", bufs=3))
        wrk = ctx.enter_context(tc.tile_pool(name="wrk", bufs=3))
        spk = ctx.enter_context(tc.tile_pool(name="spk", bufs=3))
        s2p = ctx.enter_context(tc.tile_pool(name="s2p", bufs=3))
        s5p = ctx.enter_context(tc.tile_pool(name="s5p", bufs=3))
        outp = ctx.enter_context(tc.tile_pool(name="outp", bufs=3))
        p1p = ctx.enter_context(tc.tile_pool(name="p1p", bufs=1, space="PSUM"))
        p2p = ctx.enter_context(tc.tile_pool(name="p2p", bufs=1, space="PSUM"))
        ptrp = ctx.enter_context(tc.tile_pool(name="ptrp", bufs=1, space="PSUM"))
        p4p = ctx.enter_context(tc.tile_pool(name="p4p", bufs=1, space="PSUM"))
        p5p = ctx.enter_context(tc.tile_pool(name="p5p", bufs=1, space="PSUM"))

        m0t = consts.tile([128, 128], dt.float32)
        nc.sync.dma_start(m0t[:], m0t_d[:])
        mw = consts.tile([128, 128], dt.bfloat16)
        nc.sync.dma_start(mw[:], mw_d[:])
        m02 = consts.tile([64, 64], dt.bfloat16)
        nc.sync.dma_start(m02[:], m02_d[:])
        mw2 = consts.tile([64, 64], dt.bfloat16)
        nc.sync.dma_start(mw2[:], mw2_d[:])
        ident = consts.tile([64, 64], dt.bfloat16)
        nc.sync.dma_start(ident[:], id_d[:])

        z1 = state.tile([128, FB], dt.float32, tag="z1")
        z2 = state.tile([128, FB], dt.float32, tag="z2")
        z3 = state.tile([64, FS], dt.float32, tag="z3")
        z4 = state.tile([64, FS], dt.float32, tag="z4")
        z5 = state.tile([64, FS], dt.float32, tag="z5")
        for z in (z1, z2, z3, z4, z5):
            nc.vector.memset(z[:], 0.0)

        def lif(psum, z_ap, th, s_tile):
            """One LIF step: z' = A*w*(w<th), s = (w>=th) with w = psum + z."""
            P, F = psum.shape
            w = wrk.tile([P, F], dt.float32, tag=f"w{P}")
            nc.vector.tensor_tensor(w[:], psum[:], z_ap, Alu.add)
            m = wrk.tile([P, F], dt.float32, tag=f"m{P}")
            nc.vector.tensor_scalar(m[:], w[:], th, A, Alu.is_lt, Alu.mult)
            nc.vector.tensor_tensor(z_ap, w[:], m[:], Alu.mult)
            nc.gpsimd.tensor_scalar(s_tile[:], w[:], th, None, Alu.is_ge)

        s2_prev = None
        s5_prev = None
        for d in range(D):
            x_sb = xin.tile([128, FB], dt.float32)
            for b in range(B):
                nc.sync.dma_start(x_sb[:, ts(b, 128)], x_d[b, d])
            p1 = p1p.tile([128, FB], dt.float32)
            for b in range(B):
                nc.tensor.matmul(p1[:, ts(b, 128)], lhsT=x_sb[:, ts(b, 128)],
                                 rhs=m0t[:], start=True, stop=True)
            s1 = spk.tile([128, FB], dt.bfloat16, tag="s1")
            lif(p1, z1[:], VTH, s1)
            p2 = p2p.tile([128, FB], dt.float32)
            for b in range(B):
                nc.tensor.matmul(p2[:, ts(b, 128)], lhsT=mw[:],
                                 rhs=s1[:, ts(b, 128)], start=True, stop=True)
            s2 = s2p.tile([128, FB], dt.bfloat16, tag="s2")
            lif(p2, z2[:], TH2, s2)

            if d % 2 == 0:
                s2_prev = s2
                continue
            e = d // 2
            sum_r = wrk.tile([128, FB], dt.bfloat16, tag="sumr")
            nc.vector.tensor_tensor(sum_r[:], s2_prev[:], s2[:], Alu.add)
            diff_r = wrk.tile([128, FB], dt.bfloat16, tag="diffr")
            nc.vector.tensor_tensor(diff_r[:], s2_prev[:], s2[:], Alu.subtract)
            o_sum = outp.tile([128, FB], dt.float32, tag="osum")
            nc.scalar.activation(o_sum[:], sum_r[:], Act.Copy, 0.0, C3)
            o_diff = outp.tile([128, FB], dt.float32, tag="odiff")
            nc.scalar.activation(o_diff[:], diff_r[:], Act.Copy, 0.0, C3)
            for b in range(B):
                nc.sync.dma_start(out1_d[b, e, 0], o_sum[:, ts(b, 128)])
                nc.sync.dma_start(out1_d[b, e, 1], o_diff[:, ts(b, 128)])

            # ---- stage 2 ----
            ptr = ptrp.tile([64, FS], dt.bfloat16)
            for b in range(B):
                nc.tensor.transpose(ptr[:, ts(b, 64)],
                                    in_=sum_r[0:64, b * 128:b * 128 + 64],
                                    identity=ident[:])
            s3 = spk.tile([64, FS], dt.bfloat16, tag="s3")
            lif(ptr, z3[:], TH2, s3)
            p4 = p4p.tile([64, FS], dt.float32)
            for b in range(B):
                nc.tensor.matmul(p4[:, ts(b, 64)], lhsT=s3[:, ts(b, 64)],
                                 rhs=m02[:], start=True, stop=True)
            s4 = spk.tile([64, FS], dt.bfloat16, tag="s4")
            lif(p4, z4[:], TH2, s4)
            p5 = p5p.tile([64, FS], dt.float32)
            for b in range(B):
                nc.tensor.matmul(p5[:, ts(b, 64)], lhsT=mw2[:],
                                 rhs=s4[:, ts(b, 64)], start=True, stop=True)
            s5 = s5p.tile([64, FS], dt.bfloat16, tag="s5")
            lif(p5, z5[:], TH2, s5)

            if e % 2 == 0:
                s5_prev = s5
                continue
            f = e // 2
            sum5 = wrk.tile([64, FS], dt.bfloat16, tag="sum5")
            nc.vector.tensor_tensor(sum5[:], s5_prev[:], s5[:], Alu.add)
            diff5 = wrk.tile([64, FS], dt.bfloat16, tag="diff5")
            nc.vector.tensor_tensor(diff5[:], s5_prev[:], s5[:], Alu.subtract)
            o2s = outp.tile([64, FS], dt.float32, tag="o2s")
            nc.scalar.activation(o2s[:], sum5[:], Act.Copy, 0.0, C3)
            o2d = outp.tile([64, FS], dt.float32, tag="o2d")
            nc.scalar.activation(o2d[:], diff5[:], Act.Copy, 0.0, C3)
            for b in range(B):
                nc.sync.dma_start(out2_d[b, f, 0], o2s[:, ts(b, 64)])
                nc.sync.dma_start(out2_d[b, f, 1], o2d[:, ts(b, 64)])

    nc.compile()
    return nc


def _get_nc():
    if "nc" not in _CACHE:
        _CACHE["nc"] = _build()
    return _CACHE["nc"]


def _assemble(o1, o2):
    """o1: (64, E1, 2, 128, 128); o2: (64, E2, 2, 64, 64) float32."""
    o1 = o1.reshape(N, C, E1, 2, 128, 128)
    o2 = o2.reshape(N, C, E2, 2, 64, 64)

    def tq(tile, qs, ps):
        # tile[..., q, p] -> [..., p, q]
        return np.swapaxes(tile[..., qs, ps], -1, -2)

    st, df = o1[:, :, :, 0], o1[:, :, :, 1]
    qL, qH = slice(0, 64), slice(64, 128)
    high1 = np.stack([
        tq(st, qH, qL), tq(st, qL, qH), tq(st, qH, qH),
        tq(df, qL, qL), tq(df, qH, qL), tq(df, qL, qH), tq(df, qH, qH),
    ], axis=1)
    high1 = np.ascontiguousarray(np.transpose(high1, (0, 1, 2, 3, 4, 5)))
    # axes currently (N, k, C, e, p, q) after stack? stack gives (N, 7, C, e, p, q)
    st2, df2 = o2[:, :, :, 0], o2[:, :, :, 1]
    l, h = slice(0, 32), slice(32, 64)
    high2 = np.stack([
        tq(st2, h, l), tq(st2, l, h), tq(st2, h, h),
        tq(df2, l, l), tq(df2, h, l), tq(df2, l, h), tq(df2, h, h),
    ], axis=1)
    sub2_0 = tq(st2, l, l)
    return (np.ascontiguousarray(sub2_0),
            np.ascontiguousarray(high1),
            np.ascontiguousarray(high2))


def kernel(x):
    from concourse.bass_utils import run_bass_kernel_spmd
    x = np.asarray(x, dtype=np.float32)
    nc = _get_nc()
    consts = _constants()
    xf = x.reshape(N * C, D, H, W)
    in_maps = []
    for k in range(NCORES):
        m = {"x": np.ascontiguousarray(xf[k * B:(k + 1) * B])}
        m.update(consts)
        in_maps.append(m)
    res = run_bass_kernel_spmd(nc, in_maps, core_ids=list(range(NCORES)))
    o1 = np.concatenate([r["out1"] for r in res.results], axis=0)
    o2 = np.concatenate([r["out2"] for r in res.results], axis=0)
    return _assemble(o1, o2)


# revision 8
# speedup vs baseline: 3.6714x; 3.6714x over previous
"""Trainium2 Bass kernel for 2-level 3D Haar DWT + LIF spiking (nn_DWT_3D).

Input:  x (2, 32, 64, 128, 128) fp32  =  (N, C, D, H, W)
Output: (sub2_0, high1, high2)
  sub2_0: (2, 32, 16, 32, 32)
  high1:  (2, 7, 32, 32, 64, 64)
  high2:  (2, 7, 32, 16, 32, 32)

Strategy: data-parallel over the N*C = 64 (n, c) slices, 8 per NeuronCore.
Per core, a depth-major stream: PE does the Haar H/W transforms as matmuls
(stage-1 H-matrix carries the 1/(sqrt(2)*tau) constant; the spike-domain
matmuls use exact +-1 bf16 matrices with the scale folded into the LIF
thresholds), DVE runs the sequential LIF recurrence, GPSIMD computes the
spike comparisons, ACT applies the output scaling.
"""
import numpy as np

N, C, D, H, W = 2, 32, 64, 128, 128
NCORES = 8
B = (N * C) // NCORES          # 8 (n, c) slices per core
E1 = D // 2                    # 32
E2 = D // 4                    # 16

TAU = 1.75
VTH = 0.35
A = float(np.float32(1.0 - 1.0 / TAU))
C1 = float(np.float32(0.7071067811865476 / TAU))
C3 = float(np.float32(0.7071067811865476))
TH2 = float(np.float32(VTH * TAU * np.sqrt(2.0)))

_CACHE = {}


def _constants():
    import ml_dtypes
    bf16 = ml_dtypes.bfloat16
    M0T = np.zeros((128, 128), np.float32)   # [h, p]
    for p in range(64):
        M0T[2 * p, p] = C1
        M0T[2 * p + 1, p] = C1
        M0T[2 * p, 64 + p] = C1
        M0T[2 * p + 1, 64 + p] = -C1
    Mw = np.zeros((128, 128), bf16)          # [w, q]
    for q in range(64):
        Mw[2 * q, q] = 1.0
        Mw[2 * q + 1, q] = 1.0
        Mw[2 * q, 64 + q] = 1.0
        Mw[2 * q + 1, 64 + q] = -1.0
    M02 = np.zeros((64, 64), bf16)           # [p, p2]
    for p2 in range(32):
        M02[2 * p2, p2] = 1.0
        M02[2 * p2 + 1, p2] = 1.0
        M02[2 * p2, 32 + p2] = 1.0
        M02[2 * p2 + 1, 32 + p2] = -1.0
    Mw2 = np.zeros((64, 64), bf16)           # [q, q2]
    for q2 in range(32):
        Mw2[2 * q2, q2] = 1.0
        Mw2[2 * q2 + 1, q2] = 1.0
        Mw2[2 * q2, 32 + q2] = 1.0
        Mw2[2 * q2 + 1, 32 + q2] = -1.0
    ident = np.eye(64, dtype=bf16)
    return {"m0t": M0T, "mw": Mw, "m02": M02, "mw2": Mw2, "ident": ident}


def _register_lif_op():
    """Fused LIF state update as a custom DVE op:
    w_new = psum + select(w_old < th, w_old * A, 0)   (one instr per step)."""
    from concourse import dve_ops
    from concourse.dve_spec import Spec, Src0, Src1, C0, C1, Zero, select, lower
    from concourse.dve_spec import _has_src1 as has_src1
    from concourse.dve_uop import DveOpSpec
    for o in dve_ops.OPS:
        if o.name == "LIF_STEP_ANT":
            return o
    spec = Spec(
        body=Src0 + select(Src1 < C0, Src1 * C1, Zero),
        reference=lambda in0, in1, s0, s1, imm2: (
            in0 + np.where(in1 < s0, in1 * np.float32(s1), np.float32(0.0))
        ).astype(np.float32),
    )
    row = dve_ops._CUSTOM_DVE_ROW_BASE + len(dve_ops.OPS)
    shas = {}
    for ver in ("v3", "v4"):
        try:
            u = lower(spec, ver=ver)
            shas[ver] = DveOpSpec(name="LIF_STEP_ANT", opcode=row, uops=u,
                                  rd1_en=has_src1(spec)).sha(ver)
        except Exception:
            pass
    op = dve_ops.DveOp("LIF_STEP_ANT", spec, subdim=False, uops_sha=shas)
    dve_ops.OPS.append(op)
    dve_ops.CUSTOM_DVE_SPECS[op.name] = spec
    dve_ops._SUB_OPCODE_FOR_NAME[op.name] = row
    return op


def _build():
    import concourse.bacc as bacc
    import concourse.bass as bass
    import concourse.tile as tile
    import concourse.mybir as mybir

    LIF_OP = _register_lif_op()

    dt = mybir.dt
    Alu = mybir.AluOpType
    Act = mybir.ActivationFunctionType
    ts = bass.ts

    nc = bacc.Bacc("TRN2", target_bir_lowering=False, debug=False,
                   num_devices=NCORES)

    x_d = nc.dram_tensor("x", [B, D, H, W], dt.float32, kind="ExternalInput")
    m0t_d = nc.dram_tensor("m0t", [128, 128], dt.float32, kind="ExternalInput")
    mw_d = nc.dram_tensor("mw", [128, 128], dt.bfloat16, kind="ExternalInput")
    m02_d = nc.dram_tensor("m02", [64, 64], dt.bfloat16, kind="ExternalInput")
    mw2_d = nc.dram_tensor("mw2", [64, 64], dt.bfloat16, kind="ExternalInput")
    id_d = nc.dram_tensor("ident", [64, 64], dt.bfloat16, kind="ExternalInput")
    out1_d = nc.dram_tensor("out1", [B, E1, 2, 128, 128], dt.float32,
                            kind="ExternalOutput")
    out2_d = nc.dram_tensor("out2", [B, E2, 2, 64, 64], dt.float32,
                            kind="ExternalOutput")

    FB = B * 128    # 1024: stage-1 free width (all 8 slices batched)
    FS = B * 64     # 512:  stage-2 free width

    from contextlib import ExitStack
    with tile.TileContext(nc) as tc, ExitStack() as ctx:
        consts = ctx.enter_context(tc.tile_pool(name="consts", bufs=1))
        state = ctx.enter_context(tc.tile_pool(name="state", bufs=1))
        xin = ctx.enter_context(tc.tile_pool(name="xin", bufs=3))
        wrk = ctx.enter_context(tc.tile_pool(name="wrk", bufs=3))
        spk = ctx.enter_context(tc.tile_pool(name="spk", bufs=3))
        s2p = ctx.enter_context(tc.tile_pool(name="s2p", bufs=3))
        s5p = ctx.enter_context(tc.tile_pool(name="s5p", bufs=3))
        outp = ctx.enter_context(tc.tile_pool(name="outp", bufs=3))
        p1p = ctx.enter_context(tc.tile_pool(name="p1p", bufs=1, space="PSUM"))
        p2p = ctx.enter_context(tc.tile_pool(name="p2p", bufs=1, space="PSUM"))
        ptrp = ctx.enter_context(tc.tile_pool(name="ptrp", bufs=1, space="PSUM"))
        p4p = ctx.enter_context(tc.tile_pool(name="p4p", bufs=1, space="PSUM"))
        p5p = ctx.enter_context(tc.tile_pool(name="p5p", bufs=1, space="PSUM"))

        m0t = consts.tile([128, 128], dt.float32)
        nc.sync.dma_start(m0t[:], m0t_d[:])
        mw = consts.tile([128, 128], dt.bfloat16)
        nc.sync.dma_start(mw[:], mw_d[:])
        m02 = consts.tile([64, 64], dt.bfloat16)
        nc.sync.dma_start(m02[:], m02_d[:])
        mw2 = consts.tile([64, 64], dt.bfloat16)
        nc.sync.dma_start(mw2[:], mw2_d[:])
        ident = consts.tile([64, 64], dt.bfloat16)
        nc.sync.dma_start(ident[:], id_d[:])

        w1 = [state.tile([128, FB], dt.float32, tag=f"w1_{i}", name=f"w1_{i}") for i in range(2)]
        w2 = [state.tile([128, FB], dt.float32, tag=f"w2_{i}", name=f"w2_{i}") for i in range(2)]
        w3 = [state.tile([64, FS], dt.float32, tag=f"w3_{i}", name=f"w3_{i}") for i in range(2)]
        w4 = [state.tile([64, FS], dt.float32, tag=f"w4_{i}", name=f"w4_{i}") for i in range(2)]
        w5 = [state.tile([64, FS], dt.float32, tag=f"w5_{i}", name=f"w5_{i}") for i in range(2)]
        for wp in (w1, w2, w3, w4, w5):
            nc.vector.memset(wp[0][:], 0.0)

        def lif(psum, wpair, idx, th, s_tile):
            """w_new = psum + select(w_old < th, A*w_old, 0); s = w_new >= th."""
            w_old = wpair[idx % 2]
            w_new = wpair[(idx + 1) % 2]
            nc.vector._custom_dve(LIF_OP, out=w_new[:], in0=psum[:],
                                  in1=w_old[:], s0=th, s1=A)
            nc.vector.tensor_scalar(s_tile[:], w_new[:], th, None, Alu.is_ge)

        s2_prev = None
        s5_prev = None
        for d in range(D):
            x_sb = xin.tile([128, FB], dt.float32)
            for b in range(B):
                nc.sync.dma_start(x_sb[:, ts(b, 128)], x_d[b, d])
            p1 = p1p.tile([128, FB], dt.float32)
            for b in range(B):
                nc.tensor.matmul(p1[:, ts(b, 128)], lhsT=x_sb[:, ts(b, 128)],
                                 rhs=m0t[:], start=True, stop=True)
            s1 = spk.tile([128, FB], dt.bfloat16, tag="s1")
            lif(p1, w1, d, VTH, s1)
            p2 = p2p.tile([128, FB], dt.float32)
            for b in range(B):
                nc.tensor.matmul(p2[:, ts(b, 128)], lhsT=mw[:],
                                 rhs=s1[:, ts(b, 128)], start=True, stop=True)
            s2 = s2p.tile([128, FB], dt.bfloat16, tag="s2")
            lif(p2, w2, d, TH2, s2)

            if d % 2 == 0:
                s2_prev = s2
                continue
            e = d // 2
            sum_r = wrk.tile([128, FB], dt.bfloat16, tag="sumr")
            nc.vector.tensor_tensor(sum_r[:], s2_prev[:], s2[:], Alu.add)
            diff_r = wrk.tile([128, FB], dt.bfloat16, tag="diffr")
            nc.vector.tensor_tensor(diff_r[:], s2_prev[:], s2[:], Alu.subtract)
            o_sum = outp.tile([128, FB], dt.float32, tag="osum")
            nc.scalar.activation(o_sum[:], sum_r[:], Act.Copy, 0.0, C3)
            o_diff = outp.tile([128, FB], dt.float32, tag="odiff")
            nc.scalar.activation(o_diff[:], diff_r[:], Act.Copy, 0.0, C3)
            for b in range(B):
                nc.sync.dma_start(out1_d[b, e, 0], o_sum[:, ts(b, 128)])
                nc.sync.dma_start(out1_d[b, e, 1], o_diff[:, ts(b, 128)])

            # ---- stage 2 ----
            ptr = ptrp.tile([64, FS], dt.bfloat16)
            for b in range(B):
                nc.tensor.transpose(ptr[:, ts(b, 64)],
                                    in_=sum_r[0:64, b * 128:b * 128 + 64],
                                    identity=ident[:])
            s3 = spk.tile([64, FS], dt.bfloat16, tag="s3")
            lif(ptr, w3, e, TH2, s3)
            p4 = p4p.tile([64, FS], dt.float32)
            for b in range(B):
                nc.tensor.matmul(p4[:, ts(b, 64)], lhsT=s3[:, ts(b, 64)],
                                 rhs=m02[:], start=True, stop=True)
            s4 = spk.tile([64, FS], dt.bfloat16, tag="s4")
            lif(p4, w4, e, TH2, s4)
            p5 = p5p.tile([64, FS], dt.float32)
            for b in range(B):
                nc.tensor.matmul(p5[:, ts(b, 64)], lhsT=mw2[:],
                                 rhs=s4[:, ts(b, 64)], start=True, stop=True)
            s5 = s5p.tile([64, FS], dt.bfloat16, tag="s5")
            lif(p5, w5, e, TH2, s5)

            if e % 2 == 0:
                s5_prev = s5
                continue
            f = e // 2
            sum5 = wrk.tile([64, FS], dt.bfloat16, tag="sum5")
            nc.vector.tensor_tensor(sum5[:], s5_prev[:], s5[:], Alu.add)
            diff5 = wrk.tile([64, FS], dt.bfloat16, tag="diff5")
            nc.vector.tensor_tensor(diff5[:], s5_prev[:], s5[:], Alu.subtract)
            o2s = outp.tile([64, FS], dt.float32, tag="o2s")
            nc.scalar.activation(o2s[:], sum5[:], Act.Copy, 0.0, C3)
            o2d = outp.tile([64, FS], dt.float32, tag="o2d")
            nc.scalar.activation(o2d[:], diff5[:], Act.Copy, 0.0, C3)
            for b in range(B):
                nc.sync.dma_start(out2_d[b, f, 0], o2s[:, ts(b, 64)])
                nc.sync.dma_start(out2_d[b, f, 1], o2d[:, ts(b, 64)])

    nc.compile()
    return nc


def _get_nc():
    if "nc" not in _CACHE:
        _CACHE["nc"] = _build()
    return _CACHE["nc"]


def _assemble(o1, o2):
    """o1: (64, E1, 2, 128, 128); o2: (64, E2, 2, 64, 64) float32."""
    o1 = o1.reshape(N, C, E1, 2, 128, 128)
    o2 = o2.reshape(N, C, E2, 2, 64, 64)

    def tq(tile, qs, ps):
        # tile[..., q, p] -> [..., p, q]
        return np.swapaxes(tile[..., qs, ps], -1, -2)

    st, df = o1[:, :, :, 0], o1[:, :, :, 1]
    qL, qH = slice(0, 64), slice(64, 128)
    high1 = np.stack([
        tq(st, qH, qL), tq(st, qL, qH), tq(st, qH, qH),
        tq(df, qL, qL), tq(df, qH, qL), tq(df, qL, qH), tq(df, qH, qH),
    ], axis=1)
    high1 = np.ascontiguousarray(np.transpose(high1, (0, 1, 2, 3, 4, 5)))
    # axes currently (N, k, C, e, p, q) after stack? stack gives (N, 7, C, e, p, q)
    st2, df2 = o2[:, :, :, 0], o2[:, :, :, 1]
    l, h = slice(0, 32), slice(32, 64)
    high2 = np.stack([
        tq(st2, h, l), tq(st2, l, h), tq(st2, h, h),
        tq(df2, l, l), tq(df2, h, l), tq(df2, l, h), tq(df2, h, h),
    ], axis=1)
    sub2_0 = tq(st2, l, l)
    return (np.ascontiguousarray(sub2_0),
            np.ascontiguousarray(high1),
            np.ascontiguousarray(high2))


def kernel(x):
    from concourse.bass_utils import run_bass_kernel_spmd
    x = np.asarray(x, dtype=np.float32)
    nc = _get_nc()
    consts = _constants()
    xf = x.reshape(N * C, D, H, W)
    in_maps = []
    for k in range(NCORES):
        m = {"x": np.ascontiguousarray(xf[k * B:(k + 1) * B])}
        m.update(consts)
        in_maps.append(m)
    res = run_bass_kernel_spmd(nc, in_maps, core_ids=list(range(NCORES)))
    o1 = np.concatenate([r["out1"] for r in res.results], axis=0)
    o2 = np.concatenate([r["out2"] for r in res.results], axis=0)
    return _assemble(o1, o2)


# revision 12
# speedup vs baseline: 5.7604x; 1.5690x over previous
"""Trainium2 Bass kernel for 2-level 3D Haar DWT + LIF spiking (nn_DWT_3D).

Input:  x (2, 32, 64, 128, 128) fp32  =  (N, C, D, H, W)
Output: (sub2_0, high1, high2)
  sub2_0: (2, 32, 16, 32, 32)
  high1:  (2, 7, 32, 32, 64, 64)
  high2:  (2, 7, 32, 16, 32, 32)

Strategy: data-parallel over the N*C = 64 (n, c) slices, 8 per NeuronCore.
Per core, a depth-major stream: PE does the Haar H/W transforms as matmuls
(stage-1 H-matrix carries the 1/(sqrt(2)*tau) constant; the spike-domain
matmuls use exact +-1 bf16 matrices with the scale folded into the LIF
thresholds), DVE runs the sequential LIF recurrence, GPSIMD computes the
spike comparisons, ACT applies the output scaling.
"""
import numpy as np

N, C, D, H, W = 2, 32, 64, 128, 128
NCORES = 8
B = (N * C) // NCORES          # 8 (n, c) slices per core
E1 = D // 2                    # 32
E2 = D // 4                    # 16

TAU = 1.75
VTH = 0.35
A = float(np.float32(1.0 - 1.0 / TAU))
C1 = float(np.float32(0.7071067811865476 / TAU))
C3 = float(np.float32(0.7071067811865476))
TH2 = float(np.float32(VTH * TAU * np.sqrt(2.0)))

_CACHE = {}


def _constants():
    import ml_dtypes
    bf16 = ml_dtypes.bfloat16
    M0T = np.zeros((128, 128), np.float32)   # [h, p]
    for p in range(64):
        M0T[2 * p, p] = C1
        M0T[2 * p + 1, p] = C1
        M0T[2 * p, 64 + p] = C1
        M0T[2 * p + 1, 64 + p] = -C1
    Mw = np.zeros((128, 128), bf16)          # [w, q]
    for q in range(64):
        Mw[2 * q, q] = 1.0
        Mw[2 * q + 1, q] = 1.0
        Mw[2 * q, 64 + q] = 1.0
        Mw[2 * q + 1, 64 + q] = -1.0
    M02 = np.zeros((64, 64), bf16)           # [p, p2]
    for p2 in range(32):
        M02[2 * p2, p2] = 1.0
        M02[2 * p2 + 1, p2] = 1.0
        M02[2 * p2, 32 + p2] = 1.0
        M02[2 * p2 + 1, 32 + p2] = -1.0
    Mw2 = np.zeros((64, 64), bf16)           # [q, q2]
    for q2 in range(32):
        Mw2[2 * q2, q2] = 1.0
        Mw2[2 * q2 + 1, q2] = 1.0
        Mw2[2 * q2, 32 + q2] = 1.0
        Mw2[2 * q2 + 1, 32 + q2] = -1.0
    ident = np.eye(64, dtype=bf16)
    return {"m0t": M0T, "mw": Mw, "m02": M02, "mw2": Mw2, "ident": ident}


def _register_lif_op():
    """Fused LIF state update as a custom DVE op:
    w_new = psum + select(w_old < th, w_old * A, 0)   (one instr per step)."""
    from concourse import dve_ops
    from concourse.dve_spec import Spec, Src0, Src1, C0, C1, Zero, select, lower
    from concourse.dve_spec import _has_src1 as has_src1
    from concourse.dve_uop import DveOpSpec
    for o in dve_ops.OPS:
        if o.name == "LIF_STEP_ANT":
            return o
    spec = Spec(
        body=Src0 + select(Src1 < C0, Src1 * C1, Zero),
        reference=lambda in0, in1, s0, s1, imm2: (
            in0 + np.where(in1 < s0, in1 * np.float32(s1), np.float32(0.0))
        ).astype(np.float32),
    )
    row = dve_ops._CUSTOM_DVE_ROW_BASE + len(dve_ops.OPS)
    shas = {}
    for ver in ("v3", "v4"):
        try:
            u = lower(spec, ver=ver)
            shas[ver] = DveOpSpec(name="LIF_STEP_ANT", opcode=row, uops=u,
                                  rd1_en=has_src1(spec)).sha(ver)
        except Exception:
            pass
    op = dve_ops.DveOp("LIF_STEP_ANT", spec, subdim=False, uops_sha=shas)
    dve_ops.OPS.append(op)
    dve_ops.CUSTOM_DVE_SPECS[op.name] = spec
    dve_ops._SUB_OPCODE_FOR_NAME[op.name] = row
    return op


def _build():
    import concourse.bacc as bacc
    import concourse.bass as bass
    import concourse.tile as tile
    import concourse.mybir as mybir

    LIF_OP = _register_lif_op()

    dt = mybir.dt
    Alu = mybir.AluOpType
    Act = mybir.ActivationFunctionType
    ts = bass.ts

    nc = bacc.Bacc("TRN2", target_bir_lowering=False, debug=False,
                   num_devices=NCORES)

    x_d = nc.dram_tensor("x", [B, D, H, W], dt.float32, kind="ExternalInput")
    m0t_d = nc.dram_tensor("m0t", [128, 128], dt.float32, kind="ExternalInput")
    mw_d = nc.dram_tensor("mw", [128, 128], dt.bfloat16, kind="ExternalInput")
    m02_d = nc.dram_tensor("m02", [64, 64], dt.bfloat16, kind="ExternalInput")
    mw2_d = nc.dram_tensor("mw2", [64, 64], dt.bfloat16, kind="ExternalInput")
    id_d = nc.dram_tensor("ident", [64, 64], dt.bfloat16, kind="ExternalInput")
    out1_d = nc.dram_tensor("out1", [B, E1, 2, 128, 128], dt.float32,
                            kind="ExternalOutput")
    out2_d = nc.dram_tensor("out2", [B, E2, 2, 64, 64], dt.float32,
                            kind="ExternalOutput")

    FB = B * 128    # 1024: stage-1 free width (all 8 slices batched)
    FS = B * 64     # 512:  stage-2 free width

    from contextlib import ExitStack
    with tile.TileContext(nc) as tc, ExitStack() as ctx:
        consts = ctx.enter_context(tc.tile_pool(name="consts", bufs=1))
        state = ctx.enter_context(tc.tile_pool(name="state", bufs=1))
        xin = ctx.enter_context(tc.tile_pool(name="xin", bufs=3))
        wrk = ctx.enter_context(tc.tile_pool(name="wrk", bufs=3))
        spk = ctx.enter_context(tc.tile_pool(name="spk", bufs=3))
        s2p = ctx.enter_context(tc.tile_pool(name="s2p", bufs=3))
        s5p = ctx.enter_context(tc.tile_pool(name="s5p", bufs=3))
        outp = ctx.enter_context(tc.tile_pool(name="outp", bufs=3))
        p1p = ctx.enter_context(tc.tile_pool(name="p1p", bufs=1, space="PSUM"))
        p2p = ctx.enter_context(tc.tile_pool(name="p2p", bufs=1, space="PSUM"))
        ptrp = ctx.enter_context(tc.tile_pool(name="ptrp", bufs=1, space="PSUM"))
        p4p = ctx.enter_context(tc.tile_pool(name="p4p", bufs=1, space="PSUM"))
        p5p = ctx.enter_context(tc.tile_pool(name="p5p", bufs=1, space="PSUM"))

        m0t = consts.tile([128, 128], dt.float32)
        nc.sync.dma_start(m0t[:], m0t_d[:])
        mw = consts.tile([128, 128], dt.bfloat16)
        nc.sync.dma_start(mw[:], mw_d[:])
        m02 = consts.tile([64, 64], dt.bfloat16)
        nc.sync.dma_start(m02[:], m02_d[:])
        mw2 = consts.tile([64, 64], dt.bfloat16)
        nc.sync.dma_start(mw2[:], mw2_d[:])
        ident = consts.tile([64, 64], dt.bfloat16)
        nc.sync.dma_start(ident[:], id_d[:])

        w1 = [state.tile([128, FB], dt.float32, tag=f"w1_{i}", name=f"w1_{i}") for i in range(2)]
        w2 = [state.tile([128, FB], dt.float32, tag=f"w2_{i}", name=f"w2_{i}") for i in range(2)]
        w3 = [state.tile([64, FS], dt.float32, tag=f"w3_{i}", name=f"w3_{i}") for i in range(2)]
        w4 = [state.tile([64, FS], dt.float32, tag=f"w4_{i}", name=f"w4_{i}") for i in range(2)]
        w5 = [state.tile([64, FS], dt.float32, tag=f"w5_{i}", name=f"w5_{i}") for i in range(2)]
        for wp in (w1, w2, w3, w4, w5):
            nc.vector.memset(wp[0][:], 0.0)

        def lif(psum, wpair, idx, th, s_tile):
            """w_new = psum + select(w_old < th, A*w_old, 0); s = w_new >= th."""
            w_old = wpair[idx % 2]
            w_new = wpair[(idx + 1) % 2]
            nc.vector._custom_dve(LIF_OP, out=w_new[:], in0=psum[:],
                                  in1=w_old[:], s0=th, s1=A)
            nc.vector.tensor_scalar(s_tile[:], w_new[:], th, None, Alu.is_ge)

        s2_prev = None
        s5_prev = None
        for d in range(D):
            x_sb = xin.tile([128, FB], dt.float32)
            nc.sync.dma_start(x_sb[:].rearrange("h (b w) -> h b w", b=B),
                              x_d[:, d].rearrange("b h w -> h b w"))
            p1 = p1p.tile([128, FB], dt.float32)
            for b in range(B):
                nc.tensor.matmul(p1[:, ts(b, 128)], lhsT=x_sb[:, ts(b, 128)],
                                 rhs=m0t[:], start=True, stop=True)
            s1 = spk.tile([128, FB], dt.bfloat16, tag="s1")
            lif(p1, w1, d, VTH, s1)
            p2 = p2p.tile([128, FB], dt.float32)
            for g in range(2):
                nc.tensor.matmul(p2[:, ts(g, 512)], lhsT=mw[:],
                                 rhs=s1[:, ts(g, 512)], start=True, stop=True)
            s2 = s2p.tile([128, FB], dt.bfloat16, tag="s2")
            lif(p2, w2, d, TH2, s2)

            if d % 2 == 0:
                s2_prev = s2
                continue
            e = d // 2
            sum_r = wrk.tile([128, FB], dt.bfloat16, tag="sumr")
            nc.vector.tensor_tensor(sum_r[:], s2_prev[:], s2[:], Alu.add)
            diff_r = wrk.tile([128, FB], dt.bfloat16, tag="diffr")
            nc.vector.tensor_tensor(diff_r[:], s2_prev[:], s2[:], Alu.subtract)
            o_sum = outp.tile([128, FB], dt.float32, tag="osum")
            nc.scalar.activation(o_sum[:], sum_r[:], Act.Copy, 0.0, C3)
            o_diff = outp.tile([128, FB], dt.float32, tag="odiff")
            nc.scalar.activation(o_diff[:], diff_r[:], Act.Copy, 0.0, C3)
            nc.sync.dma_start(out1_d[:, e, 0].rearrange("b q p -> q b p"),
                              o_sum[:].rearrange("q (b p) -> q b p", b=B))
            nc.sync.dma_start(out1_d[:, e, 1].rearrange("b q p -> q b p"),
                              o_diff[:].rearrange("q (b p) -> q b p", b=B))

            # ---- stage 2 ----
            ptr = ptrp.tile([64, FS], dt.bfloat16)
            for b in range(B):
                nc.tensor.transpose(ptr[:, ts(b, 64)],
                                    in_=sum_r[0:64, b * 128:b * 128 + 64],
                                    identity=ident[:])
            s3 = spk.tile([64, FS], dt.bfloat16, tag="s3")
            lif(ptr, w3, e, TH2, s3)
            # H2: contract p (partition of s3) with the constant matrix
            # stationary; out layout [p2, (b, q)].
            p4 = p4p.tile([64, FS], dt.float32)
            nc.tensor.matmul(p4[:], lhsT=m02[:], rhs=s3[:],
                             start=True, stop=True)
            s4 = spk.tile([64, FS], dt.bfloat16, tag="s4")
            lif(p4, w4, e, TH2, s4)
            # W2: contract q (free axis of s4) as strided Haar pairs on DVE.
            # raw5 layout: [p2, (b, k, q2)] with k=0 sum half, k=1 diff half.
            raw5 = wrk.tile([64, FS], dt.bfloat16, tag="raw5")
            s4v = s4[:].rearrange("p (b q r) -> p b q r", b=B, r=2)
            r5v = raw5[:].rearrange("p (b k q) -> p b k q", b=B, k=2)
            nc.vector.tensor_tensor(r5v[:, :, 0], s4v[:, :, :, 0],
                                    s4v[:, :, :, 1], Alu.add)
            nc.vector.tensor_tensor(r5v[:, :, 1], s4v[:, :, :, 0],
                                    s4v[:, :, :, 1], Alu.subtract)
            s5 = s5p.tile([64, FS], dt.bfloat16, tag="s5")
            lif(raw5, w5, e, TH2, s5)

            if e % 2 == 0:
                s5_prev = s5
                continue
            f = e // 2
            sum5 = wrk.tile([64, FS], dt.bfloat16, tag="sum5")
            nc.vector.tensor_tensor(sum5[:], s5_prev[:], s5[:], Alu.add)
            diff5 = wrk.tile([64, FS], dt.bfloat16, tag="diff5")
            nc.vector.tensor_tensor(diff5[:], s5_prev[:], s5[:], Alu.subtract)
            o2s = outp.tile([64, FS], dt.float32, tag="o2s")
            nc.scalar.activation(o2s[:], sum5[:], Act.Copy, 0.0, C3)
            o2d = outp.tile([64, FS], dt.float32, tag="o2d")
            nc.scalar.activation(o2d[:], diff5[:], Act.Copy, 0.0, C3)
            nc.sync.dma_start(out2_d[:, f, 0].rearrange("b q p -> q b p"),
                              o2s[:].rearrange("q (b p) -> q b p", b=B))
            nc.sync.dma_start(out2_d[:, f, 1].rearrange("b q p -> q b p"),
                              o2d[:].rearrange("q (b p) -> q b p", b=B))

    nc.compile()
    return nc


def _get_nc():
    if "nc" not in _CACHE:
        _CACHE["nc"] = _build()
    return _CACHE["nc"]


def _assemble(o1, o2):
    """o1: (64, E1, 2, 128, 128); o2: (64, E2, 2, 64, 64) float32."""
    o1 = o1.reshape(N, C, E1, 2, 128, 128)
    o2 = o2.reshape(N, C, E2, 2, 64, 64)

    def tq(tile, qs, ps):
        # tile[..., q, p] -> [..., p, q]
        return np.swapaxes(tile[..., qs, ps], -1, -2)

    st, df = o1[:, :, :, 0], o1[:, :, :, 1]
    qL, qH = slice(0, 64), slice(64, 128)
    high1 = np.stack([
        tq(st, qH, qL), tq(st, qL, qH), tq(st, qH, qH),
        tq(df, qL, qL), tq(df, qH, qL), tq(df, qL, qH), tq(df, qH, qH),
    ], axis=1)
    high1 = np.ascontiguousarray(np.transpose(high1, (0, 1, 2, 3, 4, 5)))
    # axes currently (N, k, C, e, p, q) after stack? stack gives (N, 7, C, e, p, q)
    # out2 tiles are [p2, (k2*32 + q2)] with p2 = H2-filter index on the
    # partition axis (already [p, q] order -> no transpose) and the free
    # axis split into W2-sum (cols 0:32) / W2-diff (cols 32:64) halves.
    st2, df2 = o2[:, :, :, 0], o2[:, :, :, 1]
    l, h = slice(0, 32), slice(32, 64)
    high2 = np.stack([
        st2[..., l, h], st2[..., h, l], st2[..., h, h],
        df2[..., l, l], df2[..., l, h], df2[..., h, l], df2[..., h, h],
    ], axis=1)
    sub2_0 = st2[..., l, l]
    return (np.ascontiguousarray(sub2_0),
            np.ascontiguousarray(high1),
            np.ascontiguousarray(high2))


def kernel(x):
    from concourse.bass_utils import run_bass_kernel_spmd
    x = np.asarray(x, dtype=np.float32)
    nc = _get_nc()
    consts = _constants()
    xf = x.reshape(N * C, D, H, W)
    in_maps = []
    for k in range(NCORES):
        m = {"x": np.ascontiguousarray(xf[k * B:(k + 1) * B])}
        m.update(consts)
        in_maps.append(m)
    res = run_bass_kernel_spmd(nc, in_maps, core_ids=list(range(NCORES)))
    o1 = np.concatenate([r["out1"] for r in res.results], axis=0)
    o2 = np.concatenate([r["out2"] for r in res.results], axis=0)
    return _assemble(o1, o2)
